# revision 14
# baseline (speedup 1.0000x reference)
"""Edge-parallel multi-head graph attention on 8 Trainium2 NeuronCores.

Strategy (matches the edge-parallel sharding hint):
  - Host: LPT-balance the 20000 destination nodes into 160 blocks of 128
    (8 cores x 20 blocks) so every block has ~4000 incoming edges; permute
    node ids so block b owns padded node ids [128b, 128b+128).  Edges are
    grouped by destination block and padded to G*512 per block.
  - Device (SPMD, one NEFF on 8 cores; all per-core variation is input
    data): each core projects the full K/V tables (K|V concatenated,
    [20480, 256] f32) into its private DRAM, projects Q for its own 2560
    nodes (pre-scaled by 1/sqrt(d)), then streams its edge groups:
    dma_gather of K|V rows by source id, one-hot dst matrices via
    is_equal, per-edge scores via PE expand + DVE mul/reduce, exp on
    ScalarE (softmax max-shift is skipped: scores are O(5) so exp cannot
    overflow and softmax is shift-invariant; the reference's eps term is
    negligible either way), and PE matmuls accumulate the weighted-V and
    softmax-denominator segments in PSUM per block.  Output is written
    feature-major and unpermuted on the host.
"""

import numpy as np

N = 20000
E = 640000
HID = 128
H = 8
D = 16
SCALE = D ** -0.5
EPS = 1e-8

NCORES = 8
P = 128                 # partitions / nodes per block
BPC = 20                # blocks per core
NB = NCORES * BPC       # 160 blocks
NP_PAD = NB * P         # 20480 padded nodes
NSH = BPC * P           # 2560 nodes per core shard
NIDX = 512              # edges per dma_gather call
SUB = NIDX // P         # 128-edge sub-tiles per gather
GRP = 1024              # edges per compute group (2 gathers)
GSUB = GRP // P         # sub-tiles per compute group

_COMPILED = {}          # G -> (nc, names)


# ----------------------------------------------------------------- host plan
def _build_plan(edge_index):
    import heapq

    src = np.asarray(edge_index[0]).astype(np.int64)
    dst = np.asarray(edge_index[1]).astype(np.int64)

    deg = np.bincount(dst, minlength=N)
    order = np.argsort(-deg, kind="stable")
    block_fill = np.zeros(NB, dtype=np.int64)
    node_block = np.empty(N, dtype=np.int64)
    node_slot = np.empty(N, dtype=np.int64)
    heap = [(0, b) for b in range(NB)]
    heapq.heapify(heap)
    loads = np.zeros(NB, dtype=np.int64)
    for n in order:
        while True:
            load, b = heapq.heappop(heap)
            if block_fill[b] < P:
                break
        node_block[n] = b
        node_slot[n] = block_fill[b]
        block_fill[b] += 1
        loads[b] = load + deg[n]
        if block_fill[b] < P:
            heapq.heappush(heap, (loads[b], b))
    perm = node_block * P + node_slot          # old node id -> padded id
    G = max(1, int(np.ceil(loads.max() / GRP)))
    cap = G * GRP

    new_dst = perm[dst]
    new_src = perm[src]
    blk = new_dst // P
    order_e = np.argsort(blk, kind="stable")
    es = new_src[order_e]
    ed = new_dst[order_e]
    eb = blk[order_e]

    src_pad = np.zeros((NB, cap), dtype=np.int32)
    dstloc_pad = np.full((NB, cap), -1.0, dtype=np.float32)
    starts = np.searchsorted(eb, np.arange(NB))
    ends = np.searchsorted(eb, np.arange(NB) + 1)
    for b in range(NB):
        s, e = starts[b], ends[b]
        src_pad[b, : e - s] = es[s:e]
        dstloc_pad[b, : e - s] = (ed[s:e] - b * P).astype(np.float32)
    return perm, G, src_pad, dstloc_pad


def _host_inputs(inputs, perm, G, src_pad, dstloc_pad):
    """Build the per-core input maps."""
    q = np.ascontiguousarray(np.asarray(inputs["query"], np.float32))
    k = np.ascontiguousarray(np.asarray(inputs["key"], np.float32))
    v = np.ascontiguousarray(np.asarray(inputs["value"], np.float32))
    Wq = np.asarray(inputs["Wq"], np.float32); bq = np.asarray(inputs["bq"], np.float32)
    Wk = np.asarray(inputs["Wk"], np.float32); bk = np.asarray(inputs["bk"], np.float32)
    Wv = np.asarray(inputs["Wv"], np.float32); bv = np.asarray(inputs["bv"], np.float32)
    Wo = np.asarray(inputs["Wo"], np.float32); bo = np.asarray(inputs["bo"], np.float32)

    qp = np.zeros((NP_PAD, HID), np.float32); qp[perm] = q
    kp = np.zeros((NP_PAD, HID), np.float32); kp[perm] = k
    vp = np.zeros((NP_PAD, HID), np.float32); vp[perm] = v
    kT = np.ascontiguousarray(kp.T)            # [128, 20480]
    vT = np.ascontiguousarray(vp.T)

    shared = {
        "kT": kT,
        "vT": vT,
        "wq_lhsT": np.ascontiguousarray((Wq * SCALE).T),
        "wk_lhsT": np.ascontiguousarray(Wk.T),
        "wv_lhsT": np.ascontiguousarray(Wv.T),
        "wo_lhsT": np.ascontiguousarray(Wo.T),
        "bq_row": np.ascontiguousarray((bq * SCALE).reshape(1, HID)),
        "bk_row": np.ascontiguousarray(bk.reshape(1, HID)),
        "bv_row": np.ascontiguousarray(bv.reshape(1, HID)),
        "bo_col": np.ascontiguousarray(bo.reshape(HID, 1)),
        "iota_row": np.tile(np.arange(P, dtype=np.int16)[None, :], (P, 1)),
        "iota_col": np.arange(P, dtype=np.int16).reshape(P, 1),
    }

    ngrp = BPC * G
    in_maps = []
    for c in range(NCORES):
        blocks = range(c * BPC, (c + 1) * BPC)
        # gather indices, wrapped by 16 and replicated to all 128 partitions
        W16 = NIDX // 16 + SUB
        ngath = 2 * G                       # 512-wide gathers per block
        gidx = np.empty((BPC, P, ngath * W16), np.int16)
        dstr = np.empty((BPC, 1, G * GRP), np.int16)
        for bi, b in enumerate(blocks):
            for g in range(ngath):
                flat_s = src_pad[b, g * NIDX : (g + 1) * NIDX]
                wrapped = flat_s.reshape(NIDX // 16, 16).T.astype(np.int16)  # [16, 32]
                gidx[bi, :, g * W16 : g * W16 + NIDX // 16] = np.tile(wrapped, (8, 1))
                flat_d = dstloc_pad[b, g * NIDX : (g + 1) * NIDX].astype(np.int16)
                gidx[bi, :, g * W16 + NIDX // 16 : (g + 1) * W16] = flat_d.reshape(
                    SUB, P
                ).T
            dstr[bi, 0] = dstloc_pad[b].astype(np.int16)
        qT_c = np.ascontiguousarray(qp[c * NSH : (c + 1) * NSH].T)  # [128, 2560]
        m = dict(shared)
        m["qT"] = qT_c
        m["gidx"] = gidx
        m["dstr"] = dstr
        in_maps.append(m)
    return in_maps


# ------------------------------------------------------------- device kernel
def _build_nc(G):
    from contextlib import ExitStack

    import concourse.bacc as bacc
    import concourse.bass as bass
    import concourse.mybir as mybir
    import concourse.tile as tile
    from concourse.masks import make_identity

    f32 = mybir.dt.float32
    bf16 = mybir.dt.bfloat16
    i16 = mybir.dt.int16
    AF = mybir.ActivationFunctionType
    W16 = NIDX // 16 + SUB
    NGATH = 2 * G

    nc = bacc.Bacc(
        "TRN2",
        target_bir_lowering=False,
        debug=False,
        num_devices=NCORES,
        dynamic_dma_scratch_size=65536,
        num_swdge_queues=2,
    )

    qT = nc.dram_tensor("qT", [P, NSH], f32, kind="ExternalInput").ap()
    kT = nc.dram_tensor("kT", [P, NP_PAD], f32, kind="ExternalInput").ap()
    vT = nc.dram_tensor("vT", [P, NP_PAD], f32, kind="ExternalInput").ap()
    wq = nc.dram_tensor("wq_lhsT", [P, P], f32, kind="ExternalInput").ap()
    wk = nc.dram_tensor("wk_lhsT", [P, P], f32, kind="ExternalInput").ap()
    wv = nc.dram_tensor("wv_lhsT", [P, P], f32, kind="ExternalInput").ap()
    wo = nc.dram_tensor("wo_lhsT", [P, P], f32, kind="ExternalInput").ap()
    bqr = nc.dram_tensor("bq_row", [1, P], f32, kind="ExternalInput").ap()
    bkr = nc.dram_tensor("bk_row", [1, P], f32, kind="ExternalInput").ap()
    bvr = nc.dram_tensor("bv_row", [1, P], f32, kind="ExternalInput").ap()
    boc = nc.dram_tensor("bo_col", [P, 1], f32, kind="ExternalInput").ap()
    iota_row = nc.dram_tensor("iota_row", [P, P], i16, kind="ExternalInput").ap()
    iota_col = nc.dram_tensor("iota_col", [P, 1], i16, kind="ExternalInput").ap()
    gidx = nc.dram_tensor(
        "gidx", [BPC, P, NGATH * W16], i16, kind="ExternalInput"
    ).ap()
    dstr = nc.dram_tensor("dstr", [BPC, 1, G * GRP], i16, kind="ExternalInput").ap()
    outT = nc.dram_tensor("outT", [P, NSH], f32, kind="ExternalOutput").ap()

    def ap3(t_ap, dims, extra_offset=0):
        return bass.AP(t_ap.tensor, t_ap.offset + extra_offset, dims)

    with tile.TileContext(nc) as tc, ExitStack() as ctx:
        const = ctx.enter_context(tc.tile_pool(name="const", bufs=1))
        dram = ctx.enter_context(tc.tile_pool(name="dram", bufs=1, space="DRAM"))
        pp = ctx.enter_context(tc.tile_pool(name="pp", bufs=3))
        blkp = ctx.enter_context(tc.tile_pool(name="blkp", bufs=2))
        ep = ctx.enter_context(tc.tile_pool(name="ep", bufs=4))
        psQ = ctx.enter_context(tc.tile_pool(name="psQ", bufs=2, space="PSUM"))
        psT = ctx.enter_context(tc.tile_pool(name="psT", bufs=2, space="PSUM"))
        psO = ctx.enter_context(tc.tile_pool(name="psO", bufs=2, space="PSUM"))

        kvf = dram.tile([NP_PAD, 2 * HID], bf16)

        c_wq = const.tile([P, P], f32); nc.sync.dma_start(c_wq[:], wq)
        c_wk = const.tile([P, P], f32); nc.sync.dma_start(c_wk[:], wk)
        c_wv = const.tile([P, P], f32); nc.sync.dma_start(c_wv[:], wv)
        c_wo = const.tile([P, P], f32); nc.sync.dma_start(c_wo[:], wo)
        c_bo = const.tile([P, 1], f32); nc.sync.dma_start(c_bo[:], boc)
        c_bqr = const.tile([P, P], f32)
        nc.sync.dma_start(c_bqr[:], ap3(bqr, [[0, P], [1, P]]))
        c_bkr = const.tile([P, P], f32)
        nc.sync.dma_start(c_bkr[:], ap3(bkr, [[0, P], [1, P]]))
        c_bvr = const.tile([P, P], f32)
        nc.sync.dma_start(c_bvr[:], ap3(bvr, [[0, P], [1, P]]))
        c_ir = const.tile([P, P], i16); nc.sync.dma_start(c_ir[:], iota_row)
        c_ic = const.tile([P, 1], i16); nc.sync.dma_start(c_ic[:], iota_col)
        ident = const.tile([P, P], f32)
        make_identity(nc, ident[:])
        epsc = const.tile([P, 1], f32)
        nc.gpsimd.memset(epsc[:], EPS)
        qsb = const.tile([P, BPC, P], bf16)

        ts = bass.ts

        # ------- phase P: out[n, j] via lhsT=x_tile, rhs=W; bias prefilled
        # into PSUM by ScalarE, matmul accumulates on top (start=False).
        W = 512
        for t in range(NP_PAD // W):
            for which, w_t, b_t in ((0, c_wk, c_bkr), (1, c_wv, c_bvr)):
                xin = kT if which == 0 else vT
                xt = pp.tile([P, W], f32, tag="xt")
                nc.sync.dma_start(xt[:], xin[:, ts(t, W)])
                for j in range(W // P):
                    mm = psQ.tile([P, P], f32, tag="qd")
                    nc.tensor.matmul(
                        mm[:], lhsT=xt[:, ts(j, P)], rhs=w_t[:], start=True, stop=True
                    )
                    kvh = pp.tile([P, P], bf16, tag=f"kvh{which}")
                    nc.vector.tensor_tensor(
                        out=kvh[:], in0=mm[:], in1=b_t[:], op=mybir.AluOpType.add
                    )
                    nc.sync.dma_start(
                        kvf[ts(t * (W // P) + j, P), ts(which, HID)], kvh[:]
                    )
        for t in range(NSH // W):
            xt = pp.tile([P, W], f32, tag="xt")
            nc.sync.dma_start(xt[:], qT[:, ts(t, W)])
            for j in range(W // P):
                mm = psQ.tile([P, P], f32, tag="qd")
                nc.tensor.matmul(
                    mm[:], lhsT=xt[:, ts(j, P)], rhs=c_wq[:], start=True, stop=True
                )
                nc.vector.tensor_tensor(
                    out=qsb[:, t * (W // P) + j, :],
                    in0=mm[:],
                    in1=c_bqr[:],
                    op=mybir.AluOpType.add,
                )

        # ------------------------- phase E: edge groups -------------------
        for b in range(BPC):
            idxb = blkp.tile([P, NGATH * W16], i16, tag="idxb")
            nc.scalar.dma_start(idxb[:], gidx[b])
            drb = blkp.tile([P, G * GRP], i16, tag="drb")
            row = dstr[b]
            nc.scalar.dma_start(drb[:], ap3(row, [[0, P]] + [list(row.ap[1])]))

            agg_ps = psO.tile([P, HID + H], f32, tag="aggp")   # [n, f | h]
            for g in range(G):
                first = g == 0
                last = g == G - 1

                kv = ep.tile([P, GSUB, 2 * HID], bf16, tag="kv")
                for half in range(2):
                    nc.gpsimd.dma_gather(
                        kv[:, half * SUB : (half + 1) * SUB, :],
                        kvf[:],
                        idxb[
                            :,
                            (2 * g + half) * W16 : (2 * g + half) * W16 + NIDX // 16,
                        ],
                        NIDX,
                        NIDX,
                        2 * HID,
                        queue_num=half,
                    )

                sel_en = ep.tile([P, GSUB, P], bf16, tag="sel_en")
                ir_ap = c_ir[:]
                i_ap = idxb[:]
                istep = i_ap.ap[1][0]
                nc.vector.tensor_tensor(
                    out=sel_en[:].rearrange("p (a j) e -> p a j e", a=2),
                    in0=ap3(
                        i_ap,
                        [
                            list(i_ap.ap[0]),
                            [W16 * istep, 2],
                            [istep, SUB],
                            [0, P],
                        ],
                        extra_offset=(2 * g * W16 + NIDX // 16) * istep,
                    ),
                    in1=ap3(
                        ir_ap,
                        [list(ir_ap.ap[0]), [0, 2], [0, SUB], list(ir_ap.ap[1])],
                    ),
                    op=mybir.AluOpType.is_equal,
                )
                sel_ne = ep.tile([P, GSUB, P], bf16, tag="sel_ne")
                drb_ap = drb[:]
                estep0 = drb_ap.ap[1][0]
                ic_ap = c_ic[:]
                nc.vector.tensor_tensor(
                    out=sel_ne[:],
                    in0=ap3(
                        drb_ap,
                        [list(drb_ap.ap[0]), [estep0 * P, GSUB], [estep0, P]],
                        extra_offset=g * GRP * estep0,
                    ),
                    in1=ap3(ic_ap, [list(ic_ap.ap[0]), [0, GSUB], [0, P]]),
                    op=mybir.AluOpType.is_equal,
                )

                qd_ps = psQ.tile([P, GRP], f32, tag="qd")
                for j in range(GSUB):
                    nc.tensor.matmul(
                        qd_ps[:, ts(j, P)],
                        lhsT=sel_ne[:, j, :],
                        rhs=qsb[:, b, :],
                        start=True,
                        stop=True,
                    )
                qd_sb = ep.tile([P, GRP], f32, tag="qd_sb")
                nc.scalar.copy(qd_sb[:], qd_ps[:])

                prod = ep.tile([P, GSUB, P], f32, tag="prod")
                nc.vector.tensor_tensor(
                    out=prod[:],
                    in0=qd_sb[:].rearrange("p (j e) -> p j e", j=GSUB),
                    in1=kv[:, :, 0:HID],
                    op=mybir.AluOpType.mult,
                )
                scores = ep.tile([P, GSUB * H], f32, tag="scores")
                nc.vector.reduce_sum(
                    out=scores[:],
                    in_=prod[:].rearrange("p j (h d) -> p (j h) d", d=D),
                    axis=mybir.AxisListType.X,
                )
                # combined [V-weighted | exp] tile: one agg matmul per sub-tile
                wvx = ep.tile([P, GSUB, HID + H], bf16, tag="wvx")
                wx_ap = wvx[:]
                wstep = wx_ap.ap[1][0]          # free stride of sub-tile dim
                nc.scalar.activation(
                    ap3(
                        wx_ap,
                        [list(wx_ap.ap[0]), [wstep, GSUB], [1, H]],
                        extra_offset=HID,
                    ),
                    scores[:].rearrange("p (j h) -> p j h", j=GSUB),
                    AF.Exp,
                )
                nc.vector.tensor_tensor(
                    out=wvx[:, :, 0:HID].rearrange("p j (h d) -> p j h d", d=D),
                    in0=kv[:, :, HID : 2 * HID].rearrange("p j (h d) -> p j h d", d=D),
                    in1=ap3(
                        wx_ap,
                        [list(wx_ap.ap[0]), [wstep, GSUB], [1, H], [0, D]],
                        extra_offset=HID,
                    ),
                    op=mybir.AluOpType.mult,
                )

                for j in range(GSUB):
                    nc.tensor.matmul(
                        agg_ps[:],
                        lhsT=sel_en[:, j, :],
                        rhs=wvx[:, j, :],
                        start=first and j == 0,
                        stop=last and j == GSUB - 1,
                    )

            # ---- block epilogue
            recip = ep.tile([P, H], f32, tag="recip")
            den = ep.tile([P, H], f32, tag="den")
            nc.scalar.activation(
                den[:], agg_ps[:, HID : HID + H], AF.Identity, bias=epsc[:, 0:1]
            )
            nc.vector.reciprocal(recip[:], den[:])
            outn = ep.tile([P, P], f32, tag="outn")
            r_ap = recip[:]
            nc.vector.tensor_tensor(
                out=outn[:].rearrange("p (h d) -> p h d", d=D),
                in0=agg_ps[:, 0:HID].rearrange("p (h d) -> p h d", d=D),
                in1=ap3(r_ap, list(r_ap.ap) + [[0, D]]),
                op=mybir.AluOpType.mult,
            )
            trn = psT.tile([P, P], f32, tag="tr")
            nc.tensor.transpose(trn[:], outn[:], ident[:])
            outnT = ep.tile([P, P], f32, tag="outnT")
            nc.scalar.copy(outnT[:], trn[:])
            fin_ps = psT.tile([P, P], f32, tag="tr")
            nc.tensor.matmul(fin_ps[:], lhsT=c_wo[:], rhs=outnT[:], start=True, stop=True)
            fin = ep.tile([P, P], f32, tag="fin")
            nc.scalar.activation(fin[:], fin_ps[:], AF.Identity, bias=c_bo[:, 0:1])
            nc.scalar.dma_start(outT[:, ts(b, P)], fin[:])

    nc.compile()
    return nc


# ---------------------------------------------------------------- entrypoint
def kernel(**inputs):
    from concourse import bass_utils

    perm, G, src_pad, dstloc_pad = _build_plan(inputs["edge_index"])
    in_maps = _host_inputs(inputs, perm, G, src_pad, dstloc_pad)

    if G not in _COMPILED:
        _COMPILED[G] = _build_nc(G)
    nc = _COMPILED[G]

    res = bass_utils.run_bass_kernel_spmd(nc, in_maps, core_ids=list(range(NCORES)))
    out_pad = np.concatenate(
        [np.asarray(res.results[c]["outT"]).T for c in range(NCORES)], axis=0
    )
    return np.ascontiguousarray(out_pad[perm])


# revision 15
# speedup vs baseline: 1.1824x; 1.1824x over previous
"""Edge-parallel multi-head graph attention on 8 Trainium2 NeuronCores.

Strategy (matches the edge-parallel sharding hint):
  - Host: LPT-balance the 20000 destination nodes into 160 blocks of 128
    (8 cores x 20 blocks) so every block has ~4000 incoming edges; permute
    node ids so block b owns padded node ids [128b, 128b+128).  Edges are
    grouped by destination block and padded to G*512 per block.
  - Device (SPMD, one NEFF on 8 cores; all per-core variation is input
    data): each core projects the full K/V tables (K|V concatenated,
    [20480, 256] f32) into its private DRAM, projects Q for its own 2560
    nodes (pre-scaled by 1/sqrt(d)), then streams its edge groups:
    dma_gather of K|V rows by source id, one-hot dst matrices via
    is_equal, per-edge scores via PE expand + DVE mul/reduce, exp on
    ScalarE (softmax max-shift is skipped: scores are O(5) so exp cannot
    overflow and softmax is shift-invariant; the reference's eps term is
    negligible either way), and PE matmuls accumulate the weighted-V and
    softmax-denominator segments in PSUM per block.  Output is written
    feature-major and unpermuted on the host.
"""

import numpy as np

N = 20000
E = 640000
HID = 128
H = 8
D = 16
SCALE = D ** -0.5
EPS = 1e-8

NCORES = 8
P = 128                 # partitions / nodes per block
BPC = 20                # blocks per core
NB = NCORES * BPC       # 160 blocks
NP_PAD = NB * P         # 20480 padded nodes
NSH = BPC * P           # 2560 nodes per core shard
NIDX = 512              # edges per dma_gather call
SUB = NIDX // P         # 128-edge sub-tiles per gather
GRP = 1024              # edges per compute group (2 gathers)
GSUB = GRP // P         # sub-tiles per compute group

_COMPILED = {}          # G -> (nc, names)


# ----------------------------------------------------------------- host plan
def _build_plan(edge_index):
    import heapq

    src = np.asarray(edge_index[0]).astype(np.int64)
    dst = np.asarray(edge_index[1]).astype(np.int64)

    deg = np.bincount(dst, minlength=N)
    order = np.argsort(-deg, kind="stable")
    block_fill = np.zeros(NB, dtype=np.int64)
    node_block = np.empty(N, dtype=np.int64)
    node_slot = np.empty(N, dtype=np.int64)
    heap = [(0, b) for b in range(NB)]
    heapq.heapify(heap)
    loads = np.zeros(NB, dtype=np.int64)
    for n in order:
        while True:
            load, b = heapq.heappop(heap)
            if block_fill[b] < P:
                break
        node_block[n] = b
        node_slot[n] = block_fill[b]
        block_fill[b] += 1
        loads[b] = load + deg[n]
        if block_fill[b] < P:
            heapq.heappush(heap, (loads[b], b))
    perm = node_block * P + node_slot          # old node id -> padded id
    G = max(1, int(np.ceil(loads.max() / GRP)))
    cap = G * GRP

    new_dst = perm[dst]
    new_src = perm[src]
    blk = new_dst // P
    order_e = np.argsort(blk, kind="stable")
    es = new_src[order_e]
    ed = new_dst[order_e]
    eb = blk[order_e]

    src_pad = np.zeros((NB, cap), dtype=np.int32)
    dstloc_pad = np.full((NB, cap), -1.0, dtype=np.float32)
    starts = np.searchsorted(eb, np.arange(NB))
    ends = np.searchsorted(eb, np.arange(NB) + 1)
    for b in range(NB):
        s, e = starts[b], ends[b]
        src_pad[b, : e - s] = es[s:e]
        dstloc_pad[b, : e - s] = (ed[s:e] - b * P).astype(np.float32)
    return perm, G, src_pad, dstloc_pad


def _host_inputs(inputs, perm, G, src_pad, dstloc_pad):
    """Build the per-core input maps."""
    q = np.ascontiguousarray(np.asarray(inputs["query"], np.float32))
    k = np.ascontiguousarray(np.asarray(inputs["key"], np.float32))
    v = np.ascontiguousarray(np.asarray(inputs["value"], np.float32))
    Wq = np.asarray(inputs["Wq"], np.float32); bq = np.asarray(inputs["bq"], np.float32)
    Wk = np.asarray(inputs["Wk"], np.float32); bk = np.asarray(inputs["bk"], np.float32)
    Wv = np.asarray(inputs["Wv"], np.float32); bv = np.asarray(inputs["bv"], np.float32)
    Wo = np.asarray(inputs["Wo"], np.float32); bo = np.asarray(inputs["bo"], np.float32)

    qp = np.zeros((NP_PAD, HID), np.float32); qp[perm] = q
    kp = np.zeros((NP_PAD, HID), np.float32); kp[perm] = k
    vp = np.zeros((NP_PAD, HID), np.float32); vp[perm] = v
    kT = np.ascontiguousarray(kp.T)            # [128, 20480]
    vT = np.ascontiguousarray(vp.T)

    shared = {
        "kT": kT,
        "vT": vT,
        "wq_lhsT": np.ascontiguousarray((Wq * SCALE).T),
        "wk_lhsT": np.ascontiguousarray(Wk.T),
        "wv_lhsT": np.ascontiguousarray(Wv.T),
        "wo_lhsT": np.ascontiguousarray(Wo.T),
        "bq_row": np.ascontiguousarray((bq * SCALE).reshape(1, HID)),
        "bk_row": np.ascontiguousarray(bk.reshape(1, HID)),
        "bv_row": np.ascontiguousarray(bv.reshape(1, HID)),
        "bo_col": np.ascontiguousarray(bo.reshape(HID, 1)),
        "iota_row": np.tile(np.arange(P, dtype=np.int16)[None, :], (P, 1)),
        "iota_col": np.arange(P, dtype=np.int16).reshape(P, 1),
    }

    ngrp = BPC * G
    in_maps = []
    for c in range(NCORES):
        blocks = range(c * BPC, (c + 1) * BPC)
        # gather indices, wrapped by 16 and replicated to all 128 partitions
        W16 = NIDX // 16 + SUB
        ngath = 2 * G                       # 512-wide gathers per block
        gidx = np.empty((BPC, P, ngath * W16), np.int16)
        dstr = np.empty((BPC, 1, G * GRP), np.int16)
        for bi, b in enumerate(blocks):
            for g in range(ngath):
                flat_s = src_pad[b, g * NIDX : (g + 1) * NIDX]
                wrapped = flat_s.reshape(NIDX // 16, 16).T.astype(np.int16)  # [16, 32]
                gidx[bi, :, g * W16 : g * W16 + NIDX // 16] = np.tile(wrapped, (8, 1))
                flat_d = dstloc_pad[b, g * NIDX : (g + 1) * NIDX].astype(np.int16)
                gidx[bi, :, g * W16 + NIDX // 16 : (g + 1) * W16] = flat_d.reshape(
                    SUB, P
                ).T
            dstr[bi, 0] = dstloc_pad[b].astype(np.int16)
        qT_c = np.ascontiguousarray(qp[c * NSH : (c + 1) * NSH].T)  # [128, 2560]
        m = dict(shared)
        m["qT"] = qT_c
        m["gidx"] = gidx
        m["dstr"] = dstr
        in_maps.append(m)
    return in_maps


# ------------------------------------------------------------- device kernel
def _build_nc(G):
    from contextlib import ExitStack

    import concourse.bacc as bacc
    import concourse.bass as bass
    import concourse.mybir as mybir
    import concourse.tile as tile
    from concourse.masks import make_identity

    f32 = mybir.dt.float32
    bf16 = mybir.dt.bfloat16
    i16 = mybir.dt.int16
    AF = mybir.ActivationFunctionType
    W16 = NIDX // 16 + SUB
    NGATH = 2 * G

    nc = bacc.Bacc(
        "TRN2",
        target_bir_lowering=False,
        debug=False,
        num_devices=NCORES,
        dynamic_dma_scratch_size=32768,
        num_swdge_queues=2,
    )

    qT = nc.dram_tensor("qT", [P, NSH], f32, kind="ExternalInput").ap()
    kT = nc.dram_tensor("kT", [P, NP_PAD], f32, kind="ExternalInput").ap()
    vT = nc.dram_tensor("vT", [P, NP_PAD], f32, kind="ExternalInput").ap()
    wq = nc.dram_tensor("wq_lhsT", [P, P], f32, kind="ExternalInput").ap()
    wk = nc.dram_tensor("wk_lhsT", [P, P], f32, kind="ExternalInput").ap()
    wv = nc.dram_tensor("wv_lhsT", [P, P], f32, kind="ExternalInput").ap()
    wo = nc.dram_tensor("wo_lhsT", [P, P], f32, kind="ExternalInput").ap()
    bqr = nc.dram_tensor("bq_row", [1, P], f32, kind="ExternalInput").ap()
    bkr = nc.dram_tensor("bk_row", [1, P], f32, kind="ExternalInput").ap()
    bvr = nc.dram_tensor("bv_row", [1, P], f32, kind="ExternalInput").ap()
    boc = nc.dram_tensor("bo_col", [P, 1], f32, kind="ExternalInput").ap()
    iota_row = nc.dram_tensor("iota_row", [P, P], i16, kind="ExternalInput").ap()
    iota_col = nc.dram_tensor("iota_col", [P, 1], i16, kind="ExternalInput").ap()
    gidx = nc.dram_tensor(
        "gidx", [BPC, P, NGATH * W16], i16, kind="ExternalInput"
    ).ap()
    dstr = nc.dram_tensor("dstr", [BPC, 1, G * GRP], i16, kind="ExternalInput").ap()
    outT = nc.dram_tensor("outT", [P, NSH], f32, kind="ExternalOutput").ap()

    def ap3(t_ap, dims, extra_offset=0):
        return bass.AP(t_ap.tensor, t_ap.offset + extra_offset, dims)

    with tile.TileContext(nc) as tc, ExitStack() as ctx:
        const = ctx.enter_context(tc.tile_pool(name="const", bufs=1))
        dram = ctx.enter_context(tc.tile_pool(name="dram", bufs=1, space="DRAM"))
        pp = ctx.enter_context(tc.tile_pool(name="pp", bufs=3))
        blkp = ctx.enter_context(tc.tile_pool(name="blkp", bufs=2))
        kvp = ctx.enter_context(tc.tile_pool(name="kvp", bufs=10))
        selp = ctx.enter_context(tc.tile_pool(name="selp", bufs=5))
        ep = ctx.enter_context(tc.tile_pool(name="ep", bufs=3))
        psQ = ctx.enter_context(tc.tile_pool(name="psQ", bufs=2, space="PSUM"))
        psT = ctx.enter_context(tc.tile_pool(name="psT", bufs=2, space="PSUM"))
        psO = ctx.enter_context(tc.tile_pool(name="psO", bufs=2, space="PSUM"))

        kvf = dram.tile([NP_PAD, 2 * HID], bf16)

        c_wq = const.tile([P, P], f32); nc.sync.dma_start(c_wq[:], wq)
        c_wk = const.tile([P, P], f32); nc.sync.dma_start(c_wk[:], wk)
        c_wv = const.tile([P, P], f32); nc.sync.dma_start(c_wv[:], wv)
        c_wo = const.tile([P, P], f32); nc.sync.dma_start(c_wo[:], wo)
        c_bo = const.tile([P, 1], f32); nc.sync.dma_start(c_bo[:], boc)
        c_bqr = const.tile([P, P], f32)
        nc.sync.dma_start(c_bqr[:], ap3(bqr, [[0, P], [1, P]]))
        c_bkr = const.tile([P, P], f32)
        nc.sync.dma_start(c_bkr[:], ap3(bkr, [[0, P], [1, P]]))
        c_bvr = const.tile([P, P], f32)
        nc.sync.dma_start(c_bvr[:], ap3(bvr, [[0, P], [1, P]]))
        c_ir = const.tile([P, P], i16); nc.sync.dma_start(c_ir[:], iota_row)
        c_ic = const.tile([P, 1], i16); nc.sync.dma_start(c_ic[:], iota_col)
        ident = const.tile([P, P], f32)
        make_identity(nc, ident[:])
        epsc = const.tile([P, 1], f32)
        nc.gpsimd.memset(epsc[:], EPS)
        qsb = const.tile([P, BPC, P], bf16)

        ts = bass.ts

        # ------- phase P: out[n, j] via lhsT=x_tile, rhs=W; bias prefilled
        # into PSUM by ScalarE, matmul accumulates on top (start=False).
        W = 512
        for t in range(NP_PAD // W):
            for which, w_t, b_t in ((0, c_wk, c_bkr), (1, c_wv, c_bvr)):
                xin = kT if which == 0 else vT
                xt = pp.tile([P, W], f32, tag="xt")
                nc.sync.dma_start(xt[:], xin[:, ts(t, W)])
                for j in range(W // P):
                    mm = psQ.tile([P, P], f32, tag="qd")
                    nc.tensor.matmul(
                        mm[:], lhsT=xt[:, ts(j, P)], rhs=w_t[:], start=True, stop=True
                    )
                    kvh = pp.tile([P, P], bf16, tag=f"kvh{which}")
                    nc.vector.tensor_tensor(
                        out=kvh[:], in0=mm[:], in1=b_t[:], op=mybir.AluOpType.add
                    )
                    nc.sync.dma_start(
                        kvf[ts(t * (W // P) + j, P), ts(which, HID)], kvh[:]
                    )
        for t in range(NSH // W):
            xt = pp.tile([P, W], f32, tag="xt")
            nc.sync.dma_start(xt[:], qT[:, ts(t, W)])
            for j in range(W // P):
                mm = psQ.tile([P, P], f32, tag="qd")
                nc.tensor.matmul(
                    mm[:], lhsT=xt[:, ts(j, P)], rhs=c_wq[:], start=True, stop=True
                )
                nc.vector.tensor_tensor(
                    out=qsb[:, t * (W // P) + j, :],
                    in0=mm[:],
                    in1=c_bqr[:],
                    op=mybir.AluOpType.add,
                )

        # ------------------------- phase E: edge groups -------------------
        for b in range(BPC):
            idxb = blkp.tile([P, NGATH * W16], i16, tag="idxb")
            nc.scalar.dma_start(idxb[:], gidx[b])
            drb = blkp.tile([P, G * GRP], i16, tag="drb")
            row = dstr[b]
            nc.scalar.dma_start(drb[:], ap3(row, [[0, P]] + [list(row.ap[1])]))

            agg_ps = psO.tile([P, HID + H], f32, tag="aggp")   # [n, f | h]

            # hoist all gathers + one-hot builds for the block so GpSimd and
            # DVE can run ahead of the per-group compute chains
            kvs = []
            for g in range(G):
                kv = kvp.tile([P, GSUB, 2 * HID], bf16, tag="kv")
                for half in range(2):
                    nc.gpsimd.dma_gather(
                        kv[:, half * SUB : (half + 1) * SUB, :],
                        kvf[:],
                        idxb[
                            :,
                            (2 * g + half) * W16 : (2 * g + half) * W16 + NIDX // 16,
                        ],
                        NIDX,
                        NIDX,
                        2 * HID,
                        queue_num=half,
                    )
                kvs.append(kv)
            sels = []
            for g in range(G):
                sel_en = selp.tile([P, GSUB, P], bf16, tag="sel_en")
                ir_ap = c_ir[:]
                i_ap = idxb[:]
                istep = i_ap.ap[1][0]
                nc.vector.tensor_tensor(
                    out=sel_en[:].rearrange("p (a j) e -> p a j e", a=2),
                    in0=ap3(
                        i_ap,
                        [
                            list(i_ap.ap[0]),
                            [W16 * istep, 2],
                            [istep, SUB],
                            [0, P],
                        ],
                        extra_offset=(2 * g * W16 + NIDX // 16) * istep,
                    ),
                    in1=ap3(
                        ir_ap,
                        [list(ir_ap.ap[0]), [0, 2], [0, SUB], list(ir_ap.ap[1])],
                    ),
                    op=mybir.AluOpType.is_equal,
                )
                sel_ne = selp.tile([P, GSUB, P], bf16, tag="sel_ne")
                drb_ap = drb[:]
                estep0 = drb_ap.ap[1][0]
                ic_ap = c_ic[:]
                nc.vector.tensor_tensor(
                    out=sel_ne[:],
                    in0=ap3(
                        drb_ap,
                        [list(drb_ap.ap[0]), [estep0 * P, GSUB], [estep0, P]],
                        extra_offset=g * GRP * estep0,
                    ),
                    in1=ap3(ic_ap, [list(ic_ap.ap[0]), [0, GSUB], [0, P]]),
                    op=mybir.AluOpType.is_equal,
                )
                sels.append((sel_en, sel_ne))

            for g in range(G):
                first = g == 0
                last = g == G - 1
                kv = kvs[g]
                sel_en, sel_ne = sels[g]

                qd_ps = psQ.tile([P, GRP], f32, tag="qd")
                for j in range(GSUB):
                    nc.tensor.matmul(
                        qd_ps[:, ts(j, P)],
                        lhsT=sel_ne[:, j, :],
                        rhs=qsb[:, b, :],
                        start=True,
                        stop=True,
                    )

                prod = ep.tile([P, GSUB, P], f32, tag="prod")
                nc.vector.tensor_tensor(
                    out=prod[:],
                    in0=qd_ps[:].rearrange("p (j e) -> p j e", j=GSUB),
                    in1=kv[:, :, 0:HID],
                    op=mybir.AluOpType.mult,
                )
                scores = ep.tile([P, GSUB * H], f32, tag="scores")
                nc.vector.reduce_sum(
                    out=scores[:],
                    in_=prod[:].rearrange("p j (h d) -> p (j h) d", d=D),
                    axis=mybir.AxisListType.X,
                )
                # combined [V-weighted | exp] tile: one agg matmul per sub-tile
                wvx = ep.tile([P, GSUB, HID + H], bf16, tag="wvx")
                wx_ap = wvx[:]
                wstep = wx_ap.ap[1][0]          # free stride of sub-tile dim
                nc.scalar.activation(
                    ap3(
                        wx_ap,
                        [list(wx_ap.ap[0]), [wstep, GSUB], [1, H]],
                        extra_offset=HID,
                    ),
                    scores[:].rearrange("p (j h) -> p j h", j=GSUB),
                    AF.Exp,
                )
                nc.vector.tensor_tensor(
                    out=wvx[:, :, 0:HID].rearrange("p j (h d) -> p j h d", d=D),
                    in0=kv[:, :, HID : 2 * HID].rearrange("p j (h d) -> p j h d", d=D),
                    in1=ap3(
                        wx_ap,
                        [list(wx_ap.ap[0]), [wstep, GSUB], [1, H], [0, D]],
                        extra_offset=HID,
                    ),
                    op=mybir.AluOpType.mult,
                )

                for j in range(GSUB):
                    nc.tensor.matmul(
                        agg_ps[:],
                        lhsT=sel_en[:, j, :],
                        rhs=wvx[:, j, :],
                        start=first and j == 0,
                        stop=last and j == GSUB - 1,
                    )

            # ---- block epilogue
            recip = ep.tile([P, H], f32, tag="recip")
            den = ep.tile([P, H], f32, tag="den")
            nc.scalar.activation(
                den[:], agg_ps[:, HID : HID + H], AF.Identity, bias=epsc[:, 0:1]
            )
            nc.vector.reciprocal(recip[:], den[:])
            outn = ep.tile([P, P], f32, tag="outn")
            r_ap = recip[:]
            nc.vector.tensor_tensor(
                out=outn[:].rearrange("p (h d) -> p h d", d=D),
                in0=agg_ps[:, 0:HID].rearrange("p (h d) -> p h d", d=D),
                in1=ap3(r_ap, list(r_ap.ap) + [[0, D]]),
                op=mybir.AluOpType.mult,
            )
            trn = psT.tile([P, P], f32, tag="tr")
            nc.tensor.transpose(trn[:], outn[:], ident[:])
            outnT = ep.tile([P, P], f32, tag="outnT")
            nc.scalar.copy(outnT[:], trn[:])
            fin_ps = psT.tile([P, P], f32, tag="tr")
            nc.tensor.matmul(fin_ps[:], lhsT=c_wo[:], rhs=outnT[:], start=True, stop=True)
            fin = ep.tile([P, P], f32, tag="fin")
            nc.scalar.activation(fin[:], fin_ps[:], AF.Identity, bias=c_bo[:, 0:1])
            nc.scalar.dma_start(outT[:, ts(b, P)], fin[:])

    nc.compile()
    return nc


# ---------------------------------------------------------------- entrypoint
def kernel(**inputs):
    from concourse import bass_utils

    perm, G, src_pad, dstloc_pad = _build_plan(inputs["edge_index"])
    in_maps = _host_inputs(inputs, perm, G, src_pad, dstloc_pad)

    if G not in _COMPILED:
        _COMPILED[G] = _build_nc(G)
    nc = _COMPILED[G]

    res = bass_utils.run_bass_kernel_spmd(nc, in_maps, core_ids=list(range(NCORES)))
    out_pad = np.concatenate(
        [np.asarray(res.results[c]["outT"]).T for c in range(NCORES)], axis=0
    )
    return np.ascontiguousarray(out_pad[perm])


# revision 16
# speedup vs baseline: 1.2230x; 1.0343x over previous
"""Edge-parallel multi-head graph attention on 8 Trainium2 NeuronCores.

Strategy (matches the edge-parallel sharding hint):
  - Host: LPT-balance the 20000 destination nodes into 160 blocks of 128
    (8 cores x 20 blocks) so every block has ~4000 incoming edges; permute
    node ids so block b owns padded node ids [128b, 128b+128).  Edges are
    grouped by destination block and padded to G*512 per block.
  - Device (SPMD, one NEFF on 8 cores; all per-core variation is input
    data): each core projects the full K/V tables (K|V concatenated,
    [20480, 256] f32) into its private DRAM, projects Q for its own 2560
    nodes (pre-scaled by 1/sqrt(d)), then streams its edge groups:
    dma_gather of K|V rows by source id, one-hot dst matrices via
    is_equal, per-edge scores via PE expand + DVE mul/reduce, exp on
    ScalarE (softmax max-shift is skipped: scores are O(5) so exp cannot
    overflow and softmax is shift-invariant; the reference's eps term is
    negligible either way), and PE matmuls accumulate the weighted-V and
    softmax-denominator segments in PSUM per block.  Output is written
    feature-major and unpermuted on the host.
"""

import numpy as np

N = 20000
E = 640000
HID = 128
H = 8
D = 16
SCALE = D ** -0.5
EPS = 1e-8

NCORES = 8
P = 128                 # partitions / nodes per block
BPC = 20                # blocks per core
NB = NCORES * BPC       # 160 blocks
NP_PAD = NB * P         # 20480 padded nodes
NSH = BPC * P           # 2560 nodes per core shard
NIDX = 512              # edges per dma_gather call
SUB = NIDX // P         # 128-edge sub-tiles per gather
GRP = 1024              # edges per compute group (2 gathers)
GSUB = GRP // P         # sub-tiles per compute group

_COMPILED = {}          # G -> (nc, names)


# ----------------------------------------------------------------- host plan
def _build_plan(edge_index):
    import heapq

    src = np.asarray(edge_index[0]).astype(np.int64)
    dst = np.asarray(edge_index[1]).astype(np.int64)

    deg = np.bincount(dst, minlength=N)
    order = np.argsort(-deg, kind="stable")
    block_fill = np.zeros(NB, dtype=np.int64)
    node_block = np.empty(N, dtype=np.int64)
    node_slot = np.empty(N, dtype=np.int64)
    heap = [(0, b) for b in range(NB)]
    heapq.heapify(heap)
    loads = np.zeros(NB, dtype=np.int64)
    for n in order:
        while True:
            load, b = heapq.heappop(heap)
            if block_fill[b] < P:
                break
        node_block[n] = b
        node_slot[n] = block_fill[b]
        block_fill[b] += 1
        loads[b] = load + deg[n]
        if block_fill[b] < P:
            heapq.heappush(heap, (loads[b], b))
    perm = node_block * P + node_slot          # old node id -> padded id
    G = max(1, int(np.ceil(loads.max() / GRP)))
    cap = G * GRP

    new_dst = perm[dst]
    new_src = perm[src]
    blk = new_dst // P
    order_e = np.argsort(blk, kind="stable")
    es = new_src[order_e]
    ed = new_dst[order_e]
    eb = blk[order_e]

    src_pad = np.zeros((NB, cap), dtype=np.int32)
    dstloc_pad = np.full((NB, cap), -1.0, dtype=np.float32)
    starts = np.searchsorted(eb, np.arange(NB))
    ends = np.searchsorted(eb, np.arange(NB) + 1)
    for b in range(NB):
        s, e = starts[b], ends[b]
        src_pad[b, : e - s] = es[s:e]
        dstloc_pad[b, : e - s] = (ed[s:e] - b * P).astype(np.float32)
    return perm, G, src_pad, dstloc_pad


def _host_inputs(inputs, perm, G, src_pad, dstloc_pad):
    """Build the per-core input maps."""
    q = np.ascontiguousarray(np.asarray(inputs["query"], np.float32))
    k = np.ascontiguousarray(np.asarray(inputs["key"], np.float32))
    v = np.ascontiguousarray(np.asarray(inputs["value"], np.float32))
    Wq = np.asarray(inputs["Wq"], np.float32); bq = np.asarray(inputs["bq"], np.float32)
    Wk = np.asarray(inputs["Wk"], np.float32); bk = np.asarray(inputs["bk"], np.float32)
    Wv = np.asarray(inputs["Wv"], np.float32); bv = np.asarray(inputs["bv"], np.float32)
    Wo = np.asarray(inputs["Wo"], np.float32); bo = np.asarray(inputs["bo"], np.float32)

    import ml_dtypes

    bf = ml_dtypes.bfloat16
    qp = np.zeros((NP_PAD, HID), np.float32); qp[perm] = q
    kp = np.zeros((NP_PAD, HID), np.float32); kp[perm] = k
    vp = np.zeros((NP_PAD, HID), np.float32); vp[perm] = v
    kT = np.ascontiguousarray(kp.T.astype(bf))     # [128, 20480]
    vT = np.ascontiguousarray(vp.T.astype(bf))

    # NOTE: the K projection bias bk shifts every score of a softmax segment
    # by the same amount (it only depends on (dst, head)), so it cancels in
    # softmax and is dropped.  The V bias is folded into the epilogue:
    # out += (sum_exp/(sum_exp+eps)) * bv.
    shared = {
        "kT": kT,
        "vT": vT,
        "wq_lhsT": np.ascontiguousarray((Wq * SCALE).T.astype(bf)),
        "wk_lhsT": np.ascontiguousarray(Wk.T.astype(bf)),
        "wv_lhsT": np.ascontiguousarray(Wv.T.astype(bf)),
        "wo_lhsT": np.ascontiguousarray(Wo.T),
        "bq_row": np.ascontiguousarray((bq * SCALE).reshape(1, HID)),
        "bv_row": np.ascontiguousarray(bv.reshape(1, HID)),
        "bo_col": np.ascontiguousarray(bo.reshape(HID, 1)),
        "iota_row": np.tile(np.arange(P, dtype=np.int16)[None, :], (P, 1)),
        "iota_col": np.arange(P, dtype=np.int16).reshape(P, 1),
    }

    ngrp = BPC * G
    in_maps = []
    for c in range(NCORES):
        blocks = range(c * BPC, (c + 1) * BPC)
        # gather indices, wrapped by 16 and replicated to all 128 partitions
        W16 = NIDX // 16 + SUB
        ngath = 2 * G                       # 512-wide gathers per block
        gidx = np.empty((BPC, P, ngath * W16), np.int16)
        dstr = np.empty((BPC, 1, G * GRP), np.int16)
        for bi, b in enumerate(blocks):
            for g in range(ngath):
                flat_s = src_pad[b, g * NIDX : (g + 1) * NIDX]
                wrapped = flat_s.reshape(NIDX // 16, 16).T.astype(np.int16)  # [16, 32]
                gidx[bi, :, g * W16 : g * W16 + NIDX // 16] = np.tile(wrapped, (8, 1))
                flat_d = dstloc_pad[b, g * NIDX : (g + 1) * NIDX].astype(np.int16)
                gidx[bi, :, g * W16 + NIDX // 16 : (g + 1) * W16] = flat_d.reshape(
                    SUB, P
                ).T
            dstr[bi, 0] = dstloc_pad[b].astype(np.int16)
        qT_c = np.ascontiguousarray(qp[c * NSH : (c + 1) * NSH].T.astype(bf))
        m = dict(shared)
        m["qT"] = qT_c
        m["gidx"] = gidx
        m["dstr"] = dstr
        in_maps.append(m)
    return in_maps


# ------------------------------------------------------------- device kernel
def _build_nc(G):
    from contextlib import ExitStack

    import concourse.bacc as bacc
    import concourse.bass as bass
    import concourse.mybir as mybir
    import concourse.tile as tile
    from concourse.masks import make_identity

    f32 = mybir.dt.float32
    bf16 = mybir.dt.bfloat16
    i16 = mybir.dt.int16
    AF = mybir.ActivationFunctionType
    W16 = NIDX // 16 + SUB
    NGATH = 2 * G

    nc = bacc.Bacc(
        "TRN2",
        target_bir_lowering=False,
        debug=False,
        num_devices=NCORES,
        dynamic_dma_scratch_size=32768,
        num_swdge_queues=2,
    )

    qT = nc.dram_tensor("qT", [P, NSH], bf16, kind="ExternalInput").ap()
    kT = nc.dram_tensor("kT", [P, NP_PAD], bf16, kind="ExternalInput").ap()
    vT = nc.dram_tensor("vT", [P, NP_PAD], bf16, kind="ExternalInput").ap()
    wq = nc.dram_tensor("wq_lhsT", [P, P], bf16, kind="ExternalInput").ap()
    wk = nc.dram_tensor("wk_lhsT", [P, P], bf16, kind="ExternalInput").ap()
    wv = nc.dram_tensor("wv_lhsT", [P, P], bf16, kind="ExternalInput").ap()
    wo = nc.dram_tensor("wo_lhsT", [P, P], f32, kind="ExternalInput").ap()
    bqr = nc.dram_tensor("bq_row", [1, P], f32, kind="ExternalInput").ap()
    bvr = nc.dram_tensor("bv_row", [1, P], f32, kind="ExternalInput").ap()
    boc = nc.dram_tensor("bo_col", [P, 1], f32, kind="ExternalInput").ap()
    iota_row = nc.dram_tensor("iota_row", [P, P], i16, kind="ExternalInput").ap()
    iota_col = nc.dram_tensor("iota_col", [P, 1], i16, kind="ExternalInput").ap()
    gidx = nc.dram_tensor(
        "gidx", [BPC, P, NGATH * W16], i16, kind="ExternalInput"
    ).ap()
    dstr = nc.dram_tensor("dstr", [BPC, 1, G * GRP], i16, kind="ExternalInput").ap()
    outT = nc.dram_tensor("outT", [P, NSH], f32, kind="ExternalOutput").ap()

    def ap3(t_ap, dims, extra_offset=0):
        return bass.AP(t_ap.tensor, t_ap.offset + extra_offset, dims)

    with tile.TileContext(nc) as tc, ExitStack() as ctx:
        const = ctx.enter_context(tc.tile_pool(name="const", bufs=1))
        dram = ctx.enter_context(tc.tile_pool(name="dram", bufs=1, space="DRAM"))
        pp = ctx.enter_context(tc.tile_pool(name="pp", bufs=3))
        blkp = ctx.enter_context(tc.tile_pool(name="blkp", bufs=2))
        kvp = ctx.enter_context(tc.tile_pool(name="kvp", bufs=10))
        selp = ctx.enter_context(tc.tile_pool(name="selp", bufs=5))
        ep = ctx.enter_context(tc.tile_pool(name="ep", bufs=3))
        psQ = ctx.enter_context(tc.tile_pool(name="psQ", bufs=2, space="PSUM"))
        psT = ctx.enter_context(tc.tile_pool(name="psT", bufs=2, space="PSUM"))
        psO = ctx.enter_context(tc.tile_pool(name="psO", bufs=2, space="PSUM"))

        kvf = dram.tile([NP_PAD, 2 * HID], bf16)

        c_wq = const.tile([P, P], bf16); nc.sync.dma_start(c_wq[:], wq)
        c_wk = const.tile([P, P], bf16); nc.sync.dma_start(c_wk[:], wk)
        c_wv = const.tile([P, P], bf16); nc.sync.dma_start(c_wv[:], wv)
        c_wo = const.tile([P, P], f32); nc.sync.dma_start(c_wo[:], wo)
        c_bo = const.tile([P, 1], f32); nc.sync.dma_start(c_bo[:], boc)
        c_bqr = const.tile([P, P], f32)
        nc.sync.dma_start(c_bqr[:], ap3(bqr, [[0, P], [1, P]]))
        c_bvr = const.tile([P, P], f32)
        nc.sync.dma_start(c_bvr[:], ap3(bvr, [[0, P], [1, P]]))
        c_ir = const.tile([P, P], i16); nc.sync.dma_start(c_ir[:], iota_row)
        c_ic = const.tile([P, 1], i16); nc.sync.dma_start(c_ic[:], iota_col)
        ident = const.tile([P, P], f32)
        make_identity(nc, ident[:])
        epsc = const.tile([P, 1], f32)
        nc.gpsimd.memset(epsc[:], EPS)
        qsb = const.tile([P, BPC, P], bf16)

        ts = bass.ts

        # ------- phase P: out[n, j] via lhsT=x_tile, rhs=W; bias prefilled
        # into PSUM by ScalarE, matmul accumulates on top (start=False).
        W = 512
        for t in range(NP_PAD // W):
            for which, w_t in ((0, c_wk), (1, c_wv)):
                xin = kT if which == 0 else vT
                xt = pp.tile([P, W], bf16, tag="xt")
                nc.sync.dma_start(xt[:], xin[:, ts(t, W)])
                for j in range(W // P):
                    mm = psQ.tile([P, P], f32, tag="qd")
                    nc.tensor.matmul(
                        mm[:], lhsT=xt[:, ts(j, P)], rhs=w_t[:], start=True, stop=True
                    )
                    kvh = pp.tile([P, P], bf16, tag=f"kvh{which}")
                    nc.scalar.copy(kvh[:], mm[:])
                    nc.sync.dma_start(
                        kvf[ts(t * (W // P) + j, P), ts(which, HID)], kvh[:]
                    )
        for t in range(NSH // W):
            xt = pp.tile([P, W], bf16, tag="xt")
            nc.sync.dma_start(xt[:], qT[:, ts(t, W)])
            for j in range(W // P):
                mm = psQ.tile([P, P], f32, tag="qd")
                nc.tensor.matmul(
                    mm[:], lhsT=xt[:, ts(j, P)], rhs=c_wq[:], start=True, stop=True
                )
                nc.vector.tensor_tensor(
                    out=qsb[:, t * (W // P) + j, :],
                    in0=mm[:],
                    in1=c_bqr[:],
                    op=mybir.AluOpType.add,
                )

        # ------------------------- phase E: edge groups -------------------
        for b in range(BPC):
            idxb = blkp.tile([P, NGATH * W16], i16, tag="idxb")
            nc.scalar.dma_start(idxb[:], gidx[b])
            drb = blkp.tile([P, G * GRP], i16, tag="drb")
            row = dstr[b]
            nc.scalar.dma_start(drb[:], ap3(row, [[0, P]] + [list(row.ap[1])]))

            agg_ps = psO.tile([P, HID + H], f32, tag="aggp")   # [n, f | h]

            # hoist all gathers + one-hot builds for the block so GpSimd and
            # DVE can run ahead of the per-group compute chains
            kvs = []
            for g in range(G):
                kv = kvp.tile([P, GSUB, 2 * HID], bf16, tag="kv")
                for half in range(2):
                    nc.gpsimd.dma_gather(
                        kv[:, half * SUB : (half + 1) * SUB, :],
                        kvf[:],
                        idxb[
                            :,
                            (2 * g + half) * W16 : (2 * g + half) * W16 + NIDX // 16,
                        ],
                        NIDX,
                        NIDX,
                        2 * HID,
                        queue_num=half,
                    )
                kvs.append(kv)
            sels = []
            for g in range(G):
                sel_en = selp.tile([P, GSUB, P], bf16, tag="sel_en")
                ir_ap = c_ir[:]
                i_ap = idxb[:]
                istep = i_ap.ap[1][0]
                nc.vector.tensor_tensor(
                    out=sel_en[:].rearrange("p (a j) e -> p a j e", a=2),
                    in0=ap3(
                        i_ap,
                        [
                            list(i_ap.ap[0]),
                            [W16 * istep, 2],
                            [istep, SUB],
                            [0, P],
                        ],
                        extra_offset=(2 * g * W16 + NIDX // 16) * istep,
                    ),
                    in1=ap3(
                        ir_ap,
                        [list(ir_ap.ap[0]), [0, 2], [0, SUB], list(ir_ap.ap[1])],
                    ),
                    op=mybir.AluOpType.is_equal,
                )
                sel_ne = selp.tile([P, GSUB, P], bf16, tag="sel_ne")
                drb_ap = drb[:]
                estep0 = drb_ap.ap[1][0]
                ic_ap = c_ic[:]
                nc.vector.tensor_tensor(
                    out=sel_ne[:],
                    in0=ap3(
                        drb_ap,
                        [list(drb_ap.ap[0]), [estep0 * P, GSUB], [estep0, P]],
                        extra_offset=g * GRP * estep0,
                    ),
                    in1=ap3(ic_ap, [list(ic_ap.ap[0]), [0, GSUB], [0, P]]),
                    op=mybir.AluOpType.is_equal,
                )
                sels.append((sel_en, sel_ne))

            for g in range(G):
                first = g == 0
                last = g == G - 1
                kv = kvs[g]
                sel_en, sel_ne = sels[g]

                qd_ps = psQ.tile([P, GRP], f32, tag="qd")
                for j in range(GSUB):
                    nc.tensor.matmul(
                        qd_ps[:, ts(j, P)],
                        lhsT=sel_ne[:, j, :],
                        rhs=qsb[:, b, :],
                        start=True,
                        stop=True,
                    )

                prod = ep.tile([P, GSUB, P], f32, tag="prod")
                nc.vector.tensor_tensor(
                    out=prod[:],
                    in0=qd_ps[:].rearrange("p (j e) -> p j e", j=GSUB),
                    in1=kv[:, :, 0:HID],
                    op=mybir.AluOpType.mult,
                )
                scores = ep.tile([P, GSUB * H], f32, tag="scores")
                nc.vector.reduce_sum(
                    out=scores[:],
                    in_=prod[:].rearrange("p j (h d) -> p (j h) d", d=D),
                    axis=mybir.AxisListType.X,
                )
                # combined [V-weighted | exp] tile: one agg matmul per sub-tile
                wvx = ep.tile([P, GSUB, HID + H], bf16, tag="wvx")
                wx_ap = wvx[:]
                wstep = wx_ap.ap[1][0]          # free stride of sub-tile dim
                nc.scalar.activation(
                    ap3(
                        wx_ap,
                        [list(wx_ap.ap[0]), [wstep, GSUB], [1, H]],
                        extra_offset=HID,
                    ),
                    scores[:].rearrange("p (j h) -> p j h", j=GSUB),
                    AF.Exp,
                )
                nc.vector.tensor_tensor(
                    out=wvx[:, :, 0:HID].rearrange("p j (h d) -> p j h d", d=D),
                    in0=kv[:, :, HID : 2 * HID].rearrange("p j (h d) -> p j h d", d=D),
                    in1=ap3(
                        wx_ap,
                        [list(wx_ap.ap[0]), [wstep, GSUB], [1, H], [0, D]],
                        extra_offset=HID,
                    ),
                    op=mybir.AluOpType.mult,
                )

                for j in range(GSUB):
                    nc.tensor.matmul(
                        agg_ps[:],
                        lhsT=sel_en[:, j, :],
                        rhs=wvx[:, j, :],
                        start=first and j == 0,
                        stop=last and j == GSUB - 1,
                    )

            # ---- block epilogue
            recip = ep.tile([P, H], f32, tag="recip")
            den = ep.tile([P, H], f32, tag="den")
            nc.scalar.activation(
                den[:], agg_ps[:, HID : HID + H], AF.Identity, bias=epsc[:, 0:1]
            )
            nc.vector.reciprocal(recip[:], den[:])
            # out = (agg + sum_exp * bv) / (sum_exp + eps)   (bv folded here)
            s_ap = agg_ps[:, HID : HID + H]
            svb = ep.tile([P, P], f32, tag="svb")
            nc.vector.tensor_tensor(
                out=svb[:].rearrange("p (h d) -> p h d", d=D),
                in0=ap3(s_ap, list(s_ap.ap) + [[0, D]]),
                in1=c_bvr[:].rearrange("p (h d) -> p h d", d=D),
                op=mybir.AluOpType.mult,
            )
            aggb = ep.tile([P, P], f32, tag="aggb")
            nc.vector.tensor_tensor(
                out=aggb[:], in0=agg_ps[:, 0:HID], in1=svb[:], op=mybir.AluOpType.add
            )
            outn = ep.tile([P, P], f32, tag="outn")
            r_ap = recip[:]
            nc.vector.tensor_tensor(
                out=outn[:].rearrange("p (h d) -> p h d", d=D),
                in0=aggb[:].rearrange("p (h d) -> p h d", d=D),
                in1=ap3(r_ap, list(r_ap.ap) + [[0, D]]),
                op=mybir.AluOpType.mult,
            )
            trn = psT.tile([P, P], f32, tag="tr")
            nc.tensor.transpose(trn[:], outn[:], ident[:])
            outnT = ep.tile([P, P], f32, tag="outnT")
            nc.scalar.copy(outnT[:], trn[:])
            fin_ps = psT.tile([P, P], f32, tag="tr")
            nc.tensor.matmul(fin_ps[:], lhsT=c_wo[:], rhs=outnT[:], start=True, stop=True)
            fin = ep.tile([P, P], f32, tag="fin")
            nc.scalar.activation(fin[:], fin_ps[:], AF.Identity, bias=c_bo[:, 0:1])
            nc.scalar.dma_start(outT[:, ts(b, P)], fin[:])

    nc.compile()
    return nc


# ---------------------------------------------------------------- entrypoint
def kernel(**inputs):
    from concourse import bass_utils

    perm, G, src_pad, dstloc_pad = _build_plan(inputs["edge_index"])
    in_maps = _host_inputs(inputs, perm, G, src_pad, dstloc_pad)

    if G not in _COMPILED:
        _COMPILED[G] = _build_nc(G)
    nc = _COMPILED[G]

    res = bass_utils.run_bass_kernel_spmd(nc, in_maps, core_ids=list(range(NCORES)))
    out_pad = np.concatenate(
        [np.asarray(res.results[c]["outT"]).T for c in range(NCORES)], axis=0
    )
    return np.ascontiguousarray(out_pad[perm])


# revision 20
# speedup vs baseline: 1.5176x; 1.2409x over previous
"""Edge-parallel multi-head graph attention on 8 Trainium2 NeuronCores.

Strategy (matches the edge-parallel sharding hint):
  - Host: LPT-balance the 20000 destination nodes into 160 blocks of 128
    (8 cores x 20 blocks) so every block has ~4000 incoming edges; permute
    node ids so block b owns padded node ids [128b, 128b+128).  Edges are
    grouped by destination block and padded to G*512 per block.
  - Device (SPMD, one NEFF on 8 cores; all per-core variation is input
    data): each core projects the full K/V tables (K|V concatenated,
    [20480, 256] f32) into its private DRAM, projects Q for its own 2560
    nodes (pre-scaled by 1/sqrt(d)), then streams its edge groups:
    dma_gather of K|V rows by source id, one-hot dst matrices via
    is_equal, per-edge scores via PE expand + DVE mul/reduce, exp on
    ScalarE (softmax max-shift is skipped: scores are O(5) so exp cannot
    overflow and softmax is shift-invariant; the reference's eps term is
    negligible either way), and PE matmuls accumulate the weighted-V and
    softmax-denominator segments in PSUM per block.  Output is written
    feature-major and unpermuted on the host.
"""

import numpy as np

N = 20000
E = 640000
HID = 128
H = 8
D = 16
SCALE = D ** -0.5
EPS = 1e-8

NCORES = 8
P = 128                 # partitions / nodes per block
BPC = 20                # blocks per core
NB = NCORES * BPC       # 160 blocks
NP_PAD = NB * P         # 20480 padded nodes
NSH = BPC * P           # 2560 nodes per core shard
NIDX = 512              # edges per dma_gather call
SUB = NIDX // P         # 128-edge sub-tiles per gather
GRP = 1024              # edges per compute group (2 gathers)
GSUB = GRP // P         # sub-tiles per compute group

_COMPILED = {}          # G -> (nc, names)


# ----------------------------------------------------------------- host plan
def _build_plan(edge_index):
    import heapq

    src = np.asarray(edge_index[0]).astype(np.int64)
    dst = np.asarray(edge_index[1]).astype(np.int64)

    deg = np.bincount(dst, minlength=N)
    order = np.argsort(-deg, kind="stable")
    block_fill = np.zeros(NB, dtype=np.int64)
    node_block = np.empty(N, dtype=np.int64)
    node_slot = np.empty(N, dtype=np.int64)
    heap = [(0, b) for b in range(NB)]
    heapq.heapify(heap)
    loads = np.zeros(NB, dtype=np.int64)
    for n in order:
        while True:
            load, b = heapq.heappop(heap)
            if block_fill[b] < P:
                break
        node_block[n] = b
        node_slot[n] = block_fill[b]
        block_fill[b] += 1
        loads[b] = load + deg[n]
        if block_fill[b] < P:
            heapq.heappush(heap, (loads[b], b))
    perm = node_block * P + node_slot          # old node id -> padded id
    G = max(1, int(np.ceil(loads.max() / GRP)))
    cap = G * GRP

    new_dst = perm[dst]
    new_src = perm[src]
    blk = new_dst // P
    order_e = np.argsort(blk, kind="stable")
    es = new_src[order_e]
    ed = new_dst[order_e]
    eb = blk[order_e]

    src_pad = np.zeros((NB, cap), dtype=np.int32)
    dstloc_pad = np.full((NB, cap), -1.0, dtype=np.float32)
    starts = np.searchsorted(eb, np.arange(NB))
    ends = np.searchsorted(eb, np.arange(NB) + 1)
    for b in range(NB):
        s, e = starts[b], ends[b]
        src_pad[b, : e - s] = es[s:e]
        dstloc_pad[b, : e - s] = (ed[s:e] - b * P).astype(np.float32)
    return perm, G, src_pad, dstloc_pad


def _host_inputs(inputs, perm, G, src_pad, dstloc_pad):
    """Build the per-core input maps."""
    q = np.ascontiguousarray(np.asarray(inputs["query"], np.float32))
    k = np.ascontiguousarray(np.asarray(inputs["key"], np.float32))
    v = np.ascontiguousarray(np.asarray(inputs["value"], np.float32))
    Wq = np.asarray(inputs["Wq"], np.float32); bq = np.asarray(inputs["bq"], np.float32)
    Wk = np.asarray(inputs["Wk"], np.float32); bk = np.asarray(inputs["bk"], np.float32)
    Wv = np.asarray(inputs["Wv"], np.float32); bv = np.asarray(inputs["bv"], np.float32)
    Wo = np.asarray(inputs["Wo"], np.float32); bo = np.asarray(inputs["bo"], np.float32)

    import ml_dtypes

    bf = ml_dtypes.bfloat16
    qp = np.zeros((NP_PAD, HID), np.float32); qp[perm] = q
    kp = np.zeros((NP_PAD, HID), np.float32); kp[perm] = k
    vp = np.zeros((NP_PAD, HID), np.float32); vp[perm] = v
    kT = np.ascontiguousarray(kp.T.astype(bf))     # [128, 20480]
    vT = np.ascontiguousarray(vp.T.astype(bf))

    # NOTE: the K projection bias bk shifts every score of a softmax segment
    # by the same amount (it only depends on (dst, head)), so it cancels in
    # softmax and is dropped.  The V bias is folded into the epilogue:
    # out += (sum_exp/(sum_exp+eps)) * bv.
    shared = {
        "kT": kT,
        "vT": vT,
        "wq_lhsT": np.ascontiguousarray((Wq * SCALE).T.astype(bf)),
        "wk_lhsT": np.ascontiguousarray(Wk.T.astype(bf)),
        "wv_lhsT": np.ascontiguousarray(Wv.T.astype(bf)),
        "wo_lhsT": np.ascontiguousarray(Wo.T),
        "bq_row": np.ascontiguousarray((bq * SCALE).reshape(1, HID)),
        "bv_row": np.ascontiguousarray(bv.reshape(1, HID)),
        "bo_col": np.ascontiguousarray(bo.reshape(HID, 1)),
        "iota_row": np.tile(np.arange(P, dtype=np.int16)[None, :], (P, 1)),
        "iota_col": np.arange(P, dtype=np.int16).reshape(P, 1),
    }

    ngrp = BPC * G
    in_maps = []
    for c in range(NCORES):
        blocks = range(c * BPC, (c + 1) * BPC)
        # gather indices, wrapped by 16 and replicated to all 128 partitions
        SRCW = GRP // 16                    # 64 src-wrap cols per group
        gidx = np.empty((BPC, P, G * (SRCW + GSUB)), np.int16)
        dstr = np.empty((BPC, 1, G * GRP), np.int16)
        for bi, b in enumerate(blocks):
            for g in range(G):
                flat_s = src_pad[b, g * GRP : (g + 1) * GRP]
                wrapped = flat_s.reshape(SRCW, 16).T.astype(np.int16)   # [16, 64]
                gidx[bi, :, g * SRCW : (g + 1) * SRCW] = np.tile(wrapped, (8, 1))
                flat_d = dstloc_pad[b, g * GRP : (g + 1) * GRP].astype(np.int16)
                gidx[bi, :, G * SRCW + g * GSUB : G * SRCW + (g + 1) * GSUB] = (
                    flat_d.reshape(GSUB, P).T
                )
            dstr[bi, 0] = dstloc_pad[b].astype(np.int16)
        qT_c = np.ascontiguousarray(qp[c * NSH : (c + 1) * NSH].T.astype(bf))
        m = dict(shared)
        m["qT"] = qT_c
        m["gidx"] = gidx
        m["dstr"] = dstr
        in_maps.append(m)
    return in_maps


# ------------------------------------------------------------- device kernel
def _build_nc(G):
    from contextlib import ExitStack

    import concourse.bacc as bacc
    import concourse.bass as bass
    import concourse.mybir as mybir
    import concourse.tile as tile
    from concourse.masks import make_identity

    f32 = mybir.dt.float32
    bf16 = mybir.dt.bfloat16
    i16 = mybir.dt.int16
    AF = mybir.ActivationFunctionType
    SRCW = GRP // 16

    nc = bacc.Bacc(
        "TRN2",
        target_bir_lowering=False,
        debug=False,
        num_devices=NCORES,
        dynamic_dma_scratch_size=32768,
        num_swdge_queues=2,
    )

    qT = nc.dram_tensor("qT", [P, NSH], bf16, kind="ExternalInput").ap()
    kT = nc.dram_tensor("kT", [P, NP_PAD], bf16, kind="ExternalInput").ap()
    vT = nc.dram_tensor("vT", [P, NP_PAD], bf16, kind="ExternalInput").ap()
    wq = nc.dram_tensor("wq_lhsT", [P, P], bf16, kind="ExternalInput").ap()
    wk = nc.dram_tensor("wk_lhsT", [P, P], bf16, kind="ExternalInput").ap()
    wv = nc.dram_tensor("wv_lhsT", [P, P], bf16, kind="ExternalInput").ap()
    wo = nc.dram_tensor("wo_lhsT", [P, P], f32, kind="ExternalInput").ap()
    bqr = nc.dram_tensor("bq_row", [1, P], f32, kind="ExternalInput").ap()
    bvr = nc.dram_tensor("bv_row", [1, P], f32, kind="ExternalInput").ap()
    boc = nc.dram_tensor("bo_col", [P, 1], f32, kind="ExternalInput").ap()
    iota_row = nc.dram_tensor("iota_row", [P, P], i16, kind="ExternalInput").ap()
    iota_col = nc.dram_tensor("iota_col", [P, 1], i16, kind="ExternalInput").ap()
    gidx = nc.dram_tensor(
        "gidx", [BPC, P, G * (SRCW + GSUB)], i16, kind="ExternalInput"
    ).ap()
    dstr = nc.dram_tensor("dstr", [BPC, 1, G * GRP], i16, kind="ExternalInput").ap()
    outT = nc.dram_tensor("outT", [P, NSH], f32, kind="ExternalOutput").ap()

    def ap3(t_ap, dims, extra_offset=0):
        return bass.AP(t_ap.tensor, t_ap.offset + extra_offset, dims)

    with tile.TileContext(nc) as tc, ExitStack() as ctx:
        const = ctx.enter_context(tc.tile_pool(name="const", bufs=1))
        dram = ctx.enter_context(tc.tile_pool(name="dram", bufs=1, space="DRAM"))
        pp = ctx.enter_context(tc.tile_pool(name="pp", bufs=3))
        blkp = ctx.enter_context(tc.tile_pool(name="blkp", bufs=2))
        kvp = ctx.enter_context(tc.tile_pool(name="kvp", bufs=10))
        selp = ctx.enter_context(tc.tile_pool(name="selp", bufs=5))
        ep = ctx.enter_context(tc.tile_pool(name="ep", bufs=3))
        psQ = ctx.enter_context(tc.tile_pool(name="psQ", bufs=2, space="PSUM"))
        psT = ctx.enter_context(tc.tile_pool(name="psT", bufs=2, space="PSUM"))
        psO = ctx.enter_context(tc.tile_pool(name="psO", bufs=2, space="PSUM"))

        kvf = dram.tile([NP_PAD, 2 * HID], bf16)

        c_wq = const.tile([P, P], bf16); nc.sync.dma_start(c_wq[:], wq)
        c_wk = const.tile([P, P], bf16); nc.sync.dma_start(c_wk[:], wk)
        c_wv = const.tile([P, P], bf16); nc.sync.dma_start(c_wv[:], wv)
        c_wo = const.tile([P, P], f32); nc.sync.dma_start(c_wo[:], wo)
        c_bo = const.tile([P, 1], f32); nc.sync.dma_start(c_bo[:], boc)
        c_bqr = const.tile([P, P], f32)
        nc.sync.dma_start(c_bqr[:], ap3(bqr, [[0, P], [1, P]]))
        c_bvr = const.tile([P, P], f32)
        nc.sync.dma_start(c_bvr[:], ap3(bvr, [[0, P], [1, P]]))
        c_ir = const.tile([P, P], i16); nc.sync.dma_start(c_ir[:], iota_row)
        c_ic = const.tile([P, 1], i16); nc.sync.dma_start(c_ic[:], iota_col)
        ident = const.tile([P, P], f32)
        make_identity(nc, ident[:])
        epsc = const.tile([P, 1], f32)
        nc.gpsimd.memset(epsc[:], EPS)
        qsb = const.tile([P, BPC, P], bf16)

        ts = bass.ts

        # ------- phase P: out[n, j] via lhsT=x_tile, rhs=W; bias prefilled
        # into PSUM by ScalarE, matmul accumulates on top (start=False).
        W = 512
        JW = W // P
        for t in range(NP_PAD // W):
            for which, w_t in ((0, c_wk), (1, c_wv)):
                xin = kT if which == 0 else vT
                xt = pp.tile([P, W], bf16, tag="xt")
                nc.sync.dma_start(xt[:], xin[:, ts(t, W)])
                kvh = pp.tile([P, JW, P], bf16, tag=f"kvh{which}")
                for j in range(JW):
                    if j % 2 == 0:
                        mm = psQ.tile([P, P], f32, tag="qd", name="mmA")
                    else:
                        mm = psT.tile([P, P], f32, tag="tr", name="mmB")
                    nc.tensor.matmul(
                        mm[:], lhsT=xt[:, ts(j, P)], rhs=w_t[:], start=True, stop=True
                    )
                    if j % 2 == 0:
                        nc.scalar.copy(kvh[:, j, :], mm[:])
                    else:
                        nc.vector.tensor_copy(kvh[:, j, :], mm[:])
                kv_dst = kvf[ts(t, P * JW), ts(which, HID)]
                d_ap = kv_dst.ap
                nc.sync.dma_start(
                    bass.AP(
                        kv_dst.tensor,
                        kv_dst.offset,
                        [[d_ap[0][0], P], [d_ap[0][0] * P, JW], list(d_ap[1])],
                    ),
                    kvh[:],
                )
        for t in range(NSH // W):
            xt = pp.tile([P, W], bf16, tag="xt")
            nc.sync.dma_start(xt[:], qT[:, ts(t, W)])
            for j in range(JW):
                if j % 2 == 0:
                    mm = psQ.tile([P, P], f32, tag="qd", name="mmA")
                else:
                    mm = psT.tile([P, P], f32, tag="tr", name="mmB")
                nc.tensor.matmul(
                    mm[:], lhsT=xt[:, ts(j, P)], rhs=c_wq[:], start=True, stop=True
                )
                nc.vector.tensor_tensor(
                    out=qsb[:, t * JW + j, :],
                    in0=mm[:],
                    in1=c_bqr[:],
                    op=mybir.AluOpType.add,
                )

        # ------------------------- phase E: edge groups -------------------
        for b in range(BPC):
            idxb = blkp.tile([P, G * (SRCW + GSUB)], i16, tag="idxb")
            nc.scalar.dma_start(idxb[:], gidx[b])
            drb = blkp.tile([P, G * GRP], i16, tag="drb")
            row = dstr[b]
            nc.scalar.dma_start(drb[:], ap3(row, [[0, P]] + [list(row.ap[1])]))

            agg_ps = psO.tile([P, HID + H], f32, tag="aggp")   # [n, f | h]

            # hoist all gathers + one-hot builds for the block so GpSimd and
            # DVE can run ahead of the per-group compute chains
            kvs = []
            for g in range(G):
                kv = kvp.tile([P, GSUB, 2 * HID], bf16, tag="kv")
                nc.gpsimd.dma_gather(
                    kv[:],
                    kvf[:],
                    idxb[:, g * SRCW : (g + 1) * SRCW],
                    GRP,
                    GRP,
                    2 * HID,
                    queue_num=g % 2,
                )
                kvs.append(kv)
            sels = []
            for g in range(G):
                sel_en = selp.tile([P, GSUB, P], bf16, tag="sel_en")
                ir_ap = c_ir[:]
                i_ap = idxb[:]
                istep = i_ap.ap[1][0]
                nc.vector.tensor_tensor(
                    out=sel_en[:],
                    in0=ap3(
                        i_ap,
                        [list(i_ap.ap[0]), [istep, GSUB], [0, P]],
                        extra_offset=(G * SRCW + g * GSUB) * istep,
                    ),
                    in1=ap3(
                        ir_ap,
                        [list(ir_ap.ap[0]), [0, GSUB], list(ir_ap.ap[1])],
                    ),
                    op=mybir.AluOpType.is_equal,
                )
                sel_ne = selp.tile([P, GSUB, P], bf16, tag="sel_ne")
                drb_ap = drb[:]
                estep0 = drb_ap.ap[1][0]
                ic_ap = c_ic[:]
                nc.vector.tensor_tensor(
                    out=sel_ne[:],
                    in0=ap3(
                        drb_ap,
                        [list(drb_ap.ap[0]), [estep0 * P, GSUB], [estep0, P]],
                        extra_offset=g * GRP * estep0,
                    ),
                    in1=ap3(ic_ap, [list(ic_ap.ap[0]), [0, GSUB], [0, P]]),
                    op=mybir.AluOpType.is_equal,
                )
                sels.append((sel_en, sel_ne))

            for g in range(G):
                first = g == 0
                last = g == G - 1
                kv = kvs[g]
                sel_en, sel_ne = sels[g]

                qd_ps = psQ.tile([P, GRP], f32, tag="qd")
                for j in range(GSUB):
                    nc.tensor.matmul(
                        qd_ps[:, ts(j, P)],
                        lhsT=sel_ne[:, j, :],
                        rhs=qsb[:, b, :],
                        start=True,
                        stop=True,
                    )

                prod = ep.tile([P, GSUB, P], f32, tag="prod")
                nc.vector.tensor_tensor(
                    out=prod[:],
                    in0=qd_ps[:].rearrange("p (j e) -> p j e", j=GSUB),
                    in1=kv[:, :, 0:HID],
                    op=mybir.AluOpType.mult,
                )
                scores = ep.tile([P, GSUB * H], f32, tag="scores")
                nc.vector.reduce_sum(
                    out=scores[:],
                    in_=prod[:].rearrange("p j (h d) -> p (j h) d", d=D),
                    axis=mybir.AxisListType.X,
                )
                # combined [V-weighted | exp] tile: one agg matmul per sub-tile
                wvx = ep.tile([P, GSUB, HID + H], bf16, tag="wvx")
                wx_ap = wvx[:]
                wstep = wx_ap.ap[1][0]          # free stride of sub-tile dim
                nc.scalar.activation(
                    ap3(
                        wx_ap,
                        [list(wx_ap.ap[0]), [wstep, GSUB], [1, H]],
                        extra_offset=HID,
                    ),
                    scores[:].rearrange("p (j h) -> p j h", j=GSUB),
                    AF.Exp,
                )
                nc.vector.tensor_tensor(
                    out=wvx[:, :, 0:HID].rearrange("p j (h d) -> p j h d", d=D),
                    in0=kv[:, :, HID : 2 * HID].rearrange("p j (h d) -> p j h d", d=D),
                    in1=ap3(
                        wx_ap,
                        [list(wx_ap.ap[0]), [wstep, GSUB], [1, H], [0, D]],
                        extra_offset=HID,
                    ),
                    op=mybir.AluOpType.mult,
                )

                for j in range(GSUB):
                    nc.tensor.matmul(
                        agg_ps[:],
                        lhsT=sel_en[:, j, :],
                        rhs=wvx[:, j, :],
                        start=first and j == 0,
                        stop=last and j == GSUB - 1,
                    )

            # ---- block epilogue
            recip = ep.tile([P, H], f32, tag="recip")
            den = ep.tile([P, H], f32, tag="den")
            nc.scalar.activation(
                den[:], agg_ps[:, HID : HID + H], AF.Identity, bias=epsc[:, 0:1]
            )
            nc.vector.reciprocal(recip[:], den[:])
            # out = (agg + sum_exp * bv) / (sum_exp + eps)   (bv folded here)
            s_ap = agg_ps[:, HID : HID + H]
            svb = ep.tile([P, P], f32, tag="svb")
            nc.vector.tensor_tensor(
                out=svb[:].rearrange("p (h d) -> p h d", d=D),
                in0=ap3(s_ap, list(s_ap.ap) + [[0, D]]),
                in1=c_bvr[:].rearrange("p (h d) -> p h d", d=D),
                op=mybir.AluOpType.mult,
            )
            aggb = ep.tile([P, P], f32, tag="aggb")
            nc.vector.tensor_tensor(
                out=aggb[:], in0=agg_ps[:, 0:HID], in1=svb[:], op=mybir.AluOpType.add
            )
            outn = ep.tile([P, P], f32, tag="outn")
            r_ap = recip[:]
            nc.vector.tensor_tensor(
                out=outn[:].rearrange("p (h d) -> p h d", d=D),
                in0=aggb[:].rearrange("p (h d) -> p h d", d=D),
                in1=ap3(r_ap, list(r_ap.ap) + [[0, D]]),
                op=mybir.AluOpType.mult,
            )
            trn = psT.tile([P, P], f32, tag="tr")
            nc.tensor.transpose(trn[:], outn[:], ident[:])
            outnT = ep.tile([P, P], f32, tag="outnT")
            nc.scalar.copy(outnT[:], trn[:])
            fin_ps = psT.tile([P, P], f32, tag="tr")
            nc.tensor.matmul(fin_ps[:], lhsT=c_wo[:], rhs=outnT[:], start=True, stop=True)
            fin = ep.tile([P, P], f32, tag="fin")
            nc.scalar.activation(fin[:], fin_ps[:], AF.Identity, bias=c_bo[:, 0:1])
            nc.scalar.dma_start(outT[:, ts(b, P)], fin[:])

    nc.compile()
    return nc


# ---------------------------------------------------------------- entrypoint
def kernel(**inputs):
    from concourse import bass_utils

    perm, G, src_pad, dstloc_pad = _build_plan(inputs["edge_index"])
    in_maps = _host_inputs(inputs, perm, G, src_pad, dstloc_pad)

    if G not in _COMPILED:
        _COMPILED[G] = _build_nc(G)
    nc = _COMPILED[G]

    res = bass_utils.run_bass_kernel_spmd(nc, in_maps, core_ids=list(range(NCORES)))
    out_pad = np.concatenate(
        [np.asarray(res.results[c]["outT"]).T for c in range(NCORES)], axis=0
    )
    return np.ascontiguousarray(out_pad[perm])
